# revision 27
# baseline (speedup 1.0000x reference)
"""Bahdanau additive attention on 8 Trainium2 NeuronCores.

Math (per batch b):
    dec_f  = decoder_hidden @ W_h                     [H]
    enc_f  = encoder_outputs[b] @ W_s                 [S, H]
    energy = tanh(dec_f + enc_f) @ v + addmask        [S]
    attn   = softmax(energy)                          [S]
    context= attn @ encoder_outputs[b]                [2H]

Sharding: data-parallel over batch, 8 batches per core, weights replicated.

Device layout choice: everything runs in "transposed" space. The host
pre-transposes encoder_outputs to encT[b] = enc[b].T (shape [2H, S], bf16) so
the feature (contraction) dim of the big matmul lands on SBUF partitions.
The main matmul computes enc_f.T tiles [k=128, s=512] with W_s tiles as the
stationary operand; tanh runs on the scalar engine with dec_f as a
per-partition bias; energy = v.T @ hidden accumulates in PSUM with M=1
matmuls; softmax runs along the free dim on a single partition; attn is
broadcast across partitions with a K=1 matmul against a ones column; context
is a free-dim weighted reduction (tensor_tensor_reduce) over the same encT
tiles, so encoder_outputs is read from HBM exactly once.
"""

import numpy as np
import ml_dtypes

import concourse.bacc as bacc
import concourse.mybir as mybir
import concourse.tile as tile
from concourse.bass_utils import run_bass_kernel_spmd

# Problem shapes (hardcoded per contest rules).
B, S, H = 64, 2048, 1024
E = 2 * H            # encoder feature dim
NC = 8               # cores
BPC = B // NC        # batches per core
P = 128              # partitions
ET = E // P          # 16 e-tiles (contraction tiles of main matmul)
KT = H // P          # 8 k-tiles (hidden dim tiles)
SC = S // 512        # 4 s-chunks of 512
NEG_BIG = -1e10

F32 = mybir.dt.float32
BF16 = mybir.dt.bfloat16

_CACHE = {}


def _build():
    nc = bacc.Bacc("TRN2", target_bir_lowering=False, debug=False, num_devices=NC)

    encT_d = nc.dram_tensor("encT", [BPC, E, S], BF16, kind="ExternalInput")
    ws_d = nc.dram_tensor("wsT", [E, H], BF16, kind="ExternalInput")
    wh_d = nc.dram_tensor("whT", [H, H], BF16, kind="ExternalInput")
    dh_d = nc.dram_tensor("dhT", [H, BPC], BF16, kind="ExternalInput")
    v_d = nc.dram_tensor("vv", [P, KT], BF16, kind="ExternalInput")
    am_d = nc.dram_tensor("amask", [BPC, S], BF16, kind="ExternalInput")

    attn_d = nc.dram_tensor("attn", [BPC, S], F32, kind="ExternalOutput")
    ctx_d = nc.dram_tensor("ctxr", [P, BPC * ET], F32, kind="ExternalOutput")

    with tile.TileContext(nc) as tc:
        with (
            tc.tile_pool(name="const", bufs=1) as cpool,
            tc.tile_pool(name="psum_mm", bufs=4, space="PSUM") as mmp,
            tc.tile_pool(name="psum_en", bufs=1, space="PSUM") as enp,
        ):
            # ---- persistent constants ----
            v_sb = cpool.tile([P, KT], BF16)
            ws_sb = cpool.tile([P, ET, H], BF16)
            decf_sb = cpool.tile([P, KT, BPC], F32)
            one_one = cpool.tile([1, 1], BF16)
            nc.vector.memset(one_one[:], 1.0)
            ctx_acc = cpool.tile([P, BPC * ET], F32)

            # ---- batch pipeline ----
            with (
                tc.tile_pool(name="encp", bufs=8) as encp,
                tc.tile_pool(name="work", bufs=2) as wkp,
            ):
                # startup: dec_f = (decoder_hidden @ W_h).T -> [k, b].
                # W_h borrows one encq slot (same 16KB/partition footprint).
                # Small DMAs first so the PE's dec_f matmuls start early,
                # W_s in two k-halves so batch 0's first k-tiles aren't
                # gated on the full weight load.
                nc.sync.dma_start(v_sb[:], v_d.ap())
                wh_sb = encp.tile([P, KT, H], BF16, tag="encq")
                nc.sync.dma_start(wh_sb[:], wh_d.ap().rearrange("(t p) k -> p t k", p=P))
                dh_sb = wkp.tile([P, KT, BPC], BF16, tag="dh", bufs=1)
                nc.sync.dma_start(dh_sb[:], dh_d.ap().rearrange("(t p) b -> p t b", p=P))
                ws_ap = ws_d.ap().rearrange("(t p) k -> p t k", p=P)
                nc.sync.dma_start(ws_sb[:, :, 0:H // 2], ws_ap[:, :, 0:H // 2])

                def emit_decf(ks):
                    # dec_f matmuls. k=0 is emitted before the batch loop (its
                    # tanh consumers in batch 0 need a recorded writer);
                    # k=1..7 are injected after batch 0's first k-tile so the
                    # PE starts on the big matmul as soon as data lands.
                    # PSUM->SBUF copies go on DVE: ACT's in-order queue
                    # already holds batch-0 tanhs that consume decf_sb.
                    for k in ks:
                        dps = mmp.tile([P, BPC], F32, tag="mm", name=f"dps_{k}")
                        for h in range(KT):
                            nc.tensor.matmul(
                                dps[:],
                                wh_sb[:, h, k * P:(k + 1) * P],
                                dh_sb[:, h, :],
                                start=(h == 0),
                                stop=(h == KT - 1),
                            )
                        nc.vector.tensor_copy(decf_sb[:, k, :], dps[:])

                emit_decf([0])

                state = {}

                def emit_load(b):
                    quarters = []
                    for q in range(4):
                        encq = encp.tile([P, 4, S], BF16, tag="encq", name=f"encq_{b}_{q}")
                        nc.sync.dma_start(
                            encq[:],
                            encT_d.ap()[b, q * 512:(q + 1) * 512, :].rearrange(
                                "(t p) s -> p t s", p=P
                            ),
                        )
                        quarters.append(encq)
                    mask_st = wkp.tile([1, S], BF16, tag="mask", bufs=1, name=f"mask_{b}")
                    nc.sync.dma_start(mask_st[:], am_d.ap()[b:b + 1, :])
                    state[b] = (quarters, mask_st)

                def emit_exp(b):
                    """Exp over batch b's energy PSUM. Emitted at the start of
                    batch b+1 so the single energy-PSUM buffer frees before
                    b+1's first energy matmul needs it."""
                    energy_ps = state[b, "energy"]
                    exps = wkp.tile([1, S], F32, tag="exps", bufs=1, name=f"exps_{b}")
                    ssum = wkp.tile([1, 1], F32, tag="ssum", name=f"ssum_{b}")
                    nc.scalar.activation(
                        exps[:],
                        energy_ps[:],
                        mybir.ActivationFunctionType.Exp,
                        accum_out=ssum[:],
                    )
                    state[b, "exp"] = (exps, ssum)

                def emit_post(b, last=False):
                    """Softmax tail + attn broadcast + context for batch b.

                    Emitted in the middle of batch b+1's main loop so the PE
                    never stalls waiting on the ACT/DVE softmax chain."""
                    quarters, _ = state[b]
                    exps, ssum = state[b, "exp"]
                    sinv = wkp.tile([1, 1], F32, tag="sinv", name=f"sinv_{b}")
                    nc.vector.reciprocal(sinv[:], ssum[:])
                    attn_row = wkp.tile([1, S], F32, tag="attn_row", bufs=1,
                                        name=f"attn_row_{b}")
                    nc.vector.tensor_scalar_mul(attn_row[:], exps[:], sinv[:])
                    nc.sync.dma_start(attn_d.ap()[b:b + 1, :], attn_row[:])

                    # broadcast attn across partitions with an SWDGE DMA:
                    # re-read the row just written to DRAM with a stride-0
                    # leading dim + fp32->bf16 cast in flight
                    attn_bc = wkp.tile([P, S], BF16, tag="attn_bc", name=f"attn_bc_{b}")
                    nc.gpsimd.dma_start(
                        attn_bc[:], attn_d.ap()[b:b + 1, :].broadcast_to((P, S))
                    )

                    # context: DVE multiply + free-dim reduce. The reduce runs
                    # on DVE for pipelined batches (keeps ACT's in-order queue
                    # clear of long ops between tanh/exp), and on ACT for the
                    # final batch so the tail pipelines across two engines.
                    for e in range(ET):
                        q, qt = divmod(e, 4)
                        scr = wkp.tile([P, S], BF16, tag="scr", name=f"scr_{b}_{e}")
                        nc.vector.tensor_mul(scr[:], quarters[q][:, qt, :], attn_bc[:])
                        acc = ctx_acc[:, b * ET + e:b * ET + e + 1]
                        if last and e % 4 != 3:
                            # tail: most reduces on ACT, some on DVE, so the
                            # two engines split the trailing reduction work
                            rdummy = wkp.tile([P, 1], BF16, tag="rdummy",
                                              name=f"rdummy_{b}_{e}")
                            nc.scalar.activation(
                                rdummy.broadcast_to((P, S)),
                                scr[:],
                                mybir.ActivationFunctionType.Copy,
                                accum_out=acc,
                            )
                        else:
                            nc.vector.reduce_sum(acc, scr[:], axis=mybir.AxisListType.X)

                # batch 0's data next on the DMA queue, then the rest of W_s
                emit_load(0)
                nc.sync.dma_start(ws_sb[:, :, H // 2:H], ws_ap[:, :, H // 2:H])
                emit_load(1)
                for b in range(BPC):
                    quarters, mask_st = state[b]
                    if b > 0:
                        emit_exp(b - 1)
                    energy_ps = enp.tile([1, S], F32, tag="energy", name=f"energy_{b}")
                    state[b, "energy"] = energy_ps

                    # main matmul: enc_f.T tiles + tanh + energy accumulation.
                    # s-chunks paired inside the e-loop so each stationary
                    # W_s tile serves 2 matmuls. Energy matmuls for each half
                    # are deferred one half-iteration so the PE never waits
                    # on the tanh that produces their hidden input.
                    pending = []

                    def flush_pending():
                        for k_, sc_, hid_ in pending:
                            nc.tensor.matmul(
                                energy_ps[0:1, sc_ * 512:(sc_ + 1) * 512],
                                v_sb[:, k_:k_ + 1],
                                hid_[:],
                                start=(k_ == 0),
                                stop=False,
                                skip_group_check=True,
                            )
                        pending.clear()

                    for k in range(KT):
                        if k == 1 and b == 0:
                            emit_decf(range(1, KT))
                        if k == 2 and b > 0:
                            emit_post(b - 1)
                        if k == 4 and b + 2 < BPC:
                            emit_load(b + 2)
                        for half in range(SC // 2):
                            pss = []
                            for j in range(2):
                                sc = half * 2 + j
                                ps = mmp.tile([P, 512], F32, tag="mm",
                                              name=f"ps_{b}_{k}_{sc}")
                                pss.append(ps)
                            for e in range(ET):
                                q, qt = divmod(e, 4)
                                for j in range(2):
                                    sc = half * 2 + j
                                    nc.tensor.matmul(
                                        pss[j][:],
                                        ws_sb[:, e, k * P:(k + 1) * P],
                                        quarters[q][:, qt, sc * 512:(sc + 1) * 512],
                                        start=(e == 0),
                                        stop=(e == ET - 1),
                                    )
                            flush_pending()
                            for j in range(2):
                                sc = half * 2 + j
                                hid = wkp.tile([P, 512], BF16, tag="hid", bufs=6,
                                               name=f"hid_{b}_{k}_{sc}")
                                nc.scalar.activation(
                                    hid[:],
                                    pss[j][:],
                                    mybir.ActivationFunctionType.Tanh,
                                    bias=decf_sb[:, k, b:b + 1],
                                )
                                pending.append((k, sc, hid))
                    flush_pending()
                    # add the (0 / -1e10) mask via a K=1 matmul
                    for sc in range(SC):
                        nc.tensor.matmul(
                            energy_ps[0:1, sc * 512:(sc + 1) * 512],
                            one_one[:],
                            mask_st[0:1, sc * 512:(sc + 1) * 512],
                            start=False,
                            stop=True,
                            skip_group_check=True,
                        )
                emit_exp(BPC - 1)
                emit_post(BPC - 1, last=True)

            nc.sync.dma_start(ctx_d.ap()[:], ctx_acc[:])

    nc.compile()
    return nc


def _prep_inputs(decoder_hidden, encoder_outputs, src_mask, W_h, W_s, v):
    bf = ml_dtypes.bfloat16
    encT = np.ascontiguousarray(encoder_outputs.transpose(0, 2, 1)).astype(bf)
    wsT = W_s.astype(bf)
    whT = W_h.astype(bf)
    dhT = np.ascontiguousarray(decoder_hidden.T).astype(bf)
    vv = np.ascontiguousarray(v.reshape(KT, P).T).astype(bf)
    amask = np.where(src_mask == 0, np.float32(NEG_BIG), np.float32(0.0)).astype(bf)

    in_maps = []
    for c in range(NC):
        lo, hi = c * BPC, (c + 1) * BPC
        in_maps.append({
            "encT": np.ascontiguousarray(encT[lo:hi]),
            "wsT": wsT,
            "whT": whT,
            "dhT": np.ascontiguousarray(dhT[:, lo:hi]),
            "vv": vv,
            "amask": np.ascontiguousarray(amask[lo:hi]),
        })
    return in_maps


def kernel(decoder_hidden, encoder_outputs, src_mask, W_h, W_s, v, _trace=False):
    if "nc" not in _CACHE:
        _CACHE["nc"] = _build()
    nc = _CACHE["nc"]

    in_maps = _prep_inputs(
        np.asarray(decoder_hidden, dtype=np.float32),
        np.asarray(encoder_outputs, dtype=np.float32),
        np.asarray(src_mask),
        np.asarray(W_h, dtype=np.float32),
        np.asarray(W_s, dtype=np.float32),
        np.asarray(v, dtype=np.float32),
    )

    res = run_bass_kernel_spmd(nc, in_maps, core_ids=list(range(NC)), trace=_trace)
    _CACHE["last_result"] = res

    context = np.empty((B, E), dtype=np.float32)
    attn = np.empty((B, S), dtype=np.float32)
    for c in range(NC):
        lo, hi = c * BPC, (c + 1) * BPC
        attn[lo:hi] = res.results[c]["attn"]
        raw = res.results[c]["ctxr"]  # [P, BPC*ET]
        context[lo:hi] = raw.reshape(P, BPC, ET).transpose(1, 2, 0).reshape(BPC, E)
    return context, attn
